# revision 7
# baseline (speedup 1.0000x reference)
"""Trainium2 Bass kernel for a 3-layer tanh RNN (SEQ=512, BATCH=64, IN=128, HID=512).

Strategy (v1): data-parallel over batch — 8 NeuronCores, 8 batch rows each,
weights replicated. All compute on device:
  - per-layer input projection xw = y_prev @ W_ih^T + b   (big GEMMs, folded-T layout)
  - sequential 512-step recurrence h_t = tanh(xw_t + h_{t-1} @ W_hh^T)
Layouts keep h transposed+folded ([128 partitions, 4*BL cols] where row p of
j-block is hidden unit j*128+p) so no transposes are needed inside the scan.
Weights/activations bf16 (fp32 PSUM accumulation), xw and output fp32.
"""

import numpy as np
import ml_dtypes

S, B, IN, H, NL = 512, 64, 128, 512, 3
NCORES = 8
BL = B // NCORES          # batch rows per core
JT = H // 128             # output j-tiles
KC = H // 128             # contraction chunks
FB = JT * BL              # folded slab width per timestep

_NC_CACHE = {}


# ---------------- host-side packing ----------------

def _pack_wT(w):
    """[Hout, Hin] -> [128, (Hin/128)*(Hout/128)*128] bf16; tile (kc, j) at
    cols ((kc*JT + j)*128 : +128), lhsT[k=p, m] = w[j*128+m, kc*128+p]."""
    jt = w.shape[0] // 128
    kcn = w.shape[1] // 128
    w4 = np.asarray(w, np.float32).reshape(jt, 128, kcn, 128)
    return np.ascontiguousarray(
        w4.transpose(3, 2, 0, 1).reshape(128, kcn * jt * 128)
    ).astype(ml_dtypes.bfloat16)


def _pack_bias(b_ih, b_hh):
    """[NL, H] x2 -> [128, NL*JT] fp32, col l*JT+j row p = bias[l, j*128+p]."""
    s = np.asarray(b_ih, np.float32) + np.asarray(b_hh, np.float32)
    return np.ascontiguousarray(s.reshape(NL, JT, 128).transpose(2, 0, 1).reshape(128, NL * JT))


def _pack_xT(x_slice):
    """[S, BL, IN] -> [IN, S*BL] bf16 (t-major, b-minor columns)."""
    return np.ascontiguousarray(
        np.asarray(x_slice, np.float32).transpose(2, 0, 1).reshape(IN, -1)
    ).astype(ml_dtypes.bfloat16)


# ---------------- device program ----------------

def _build(nsteps=S, bl=BL):
    from contextlib import ExitStack
    from concourse import bacc, mybir
    import concourse.tile as tile

    f32, bf16 = mybir.dt.float32, mybir.dt.bfloat16
    Tanh = mybir.ActivationFunctionType.Tanh
    fb = JT * bl
    TBLK = min(512 // bl, nsteps)   # timesteps per GEMM n-block (N = TBLK*bl <= 512)
    NBLK = nsteps // TBLK

    nc = bacc.Bacc("TRN2", target_bir_lowering=False, debug=False, num_devices=NCORES)

    xT_d = nc.dram_tensor("xT", [IN, nsteps * bl], bf16, kind="ExternalInput")
    wih0T_d = nc.dram_tensor("wih0T", [IN, JT * 128], bf16, kind="ExternalInput")
    wihT_d = nc.dram_tensor("wihT", [128, (NL - 1) * KC * JT * 128], bf16, kind="ExternalInput")
    whhT_d = nc.dram_tensor("whhT", [128, NL * KC * JT * 128], bf16, kind="ExternalInput")
    bias_d = nc.dram_tensor("biasT", [128, NL * JT], f32, kind="ExternalInput")
    y_d = nc.dram_tensor("y_out", [nsteps, fb, 128], f32, kind="ExternalOutput")

    with tile.TileContext(nc) as tc, ExitStack() as ctx:
        consts = ctx.enter_context(tc.tile_pool(name="consts", bufs=1))
        big = ctx.enter_context(tc.tile_pool(name="big", bufs=1))
        yts = ctx.enter_context(tc.tile_pool(name="yts", bufs=2))
        gpsum = ctx.enter_context(tc.tile_pool(name="gpsum", bufs=4, space="PSUM"))
        rpsum = ctx.enter_context(tc.tile_pool(name="rpsum", bufs=4, space="PSUM"))
        small = ctx.enter_context(tc.tile_pool(name="small", bufs=6))
        ring = ctx.enter_context(tc.tile_pool(name="ring", bufs=4))

        xT_s = consts.tile([IN, nsteps * bl], bf16)
        nc.sync.dma_start(xT_s[:], xT_d.ap())
        wih0T_s = consts.tile([IN, JT * 128], bf16)
        nc.sync.dma_start(wih0T_s[:], wih0T_d.ap())
        wihT_s = consts.tile([128, (NL - 1) * KC * JT * 128], bf16)
        nc.sync.dma_start(wihT_s[:], wihT_d.ap())
        whhT_s = consts.tile([128, NL * KC * JT * 128], bf16)
        nc.sync.dma_start(whhT_s[:], whhT_d.ap())
        bias_s = consts.tile([128, NL * JT], f32)
        nc.sync.dma_start(bias_s[:], bias_d.ap())
        zeros_s = consts.tile([128, fb], bf16)
        nc.vector.memset(zeros_s[:], 0.0)

        # output DRAM folded as [t, f=(j*bl+b), p]; viewed [t, p, f] for the DMA
        y_dst = y_d.ap().rearrange("s f p -> s p f", p=128)

        yprev_r = None
        for l in range(NL):
            top = l == NL - 1
            nkc = 1 if l == 0 else KC

            # ---- input projection: xwT[p, t, j*bl+b] = (y_prev @ W_ih^T + b)[t, b, j*128+p]
            xwT = big.tile([128, nsteps * fb], f32, tag="xwT")
            xw_r = xwT[:].rearrange("p (t f) -> p t f", f=fb)
            for j in range(JT):
                for nb in range(NBLK):
                    ps = gpsum.tile([128, TBLK * bl], f32)
                    ps3 = ps[:].rearrange("p (t b) -> p t b", b=bl)
                    for kc in range(nkc):
                        if l == 0:
                            lhsT = wih0T_s[:, j * 128:(j + 1) * 128]
                            rhs = xT_s[:, nb * TBLK * bl:(nb + 1) * TBLK * bl]
                            out = ps[:]
                        else:
                            base = ((l - 1) * KC + kc) * JT + j
                            lhsT = wihT_s[:, base * 128:(base + 1) * 128]
                            rhs = yprev_r[:, nb * TBLK:(nb + 1) * TBLK, kc * bl:(kc + 1) * bl]
                            out = ps3
                        nc.tensor.matmul(out, lhsT, rhs, start=(kc == 0), stop=(kc == nkc - 1))
                    nc.vector.tensor_scalar_add(
                        out=xw_r[:, nb * TBLK:(nb + 1) * TBLK, j * bl:(j + 1) * bl],
                        in0=ps3,
                        scalar1=bias_s[:, l * JT + j:l * JT + j + 1],
                    )

            # ---- recurrence
            if not top:
                yT = yts.tile([128, nsteps * fb], bf16, tag="yT")
                y_r = yT[:].rearrange("p (t f) -> p t f", f=fb)
            prev_slab = zeros_s[:]
            for t in range(nsteps):
                ps = rpsum.tile([128, fb], f32)
                for j in range(JT):
                    for kc in range(KC):
                        base = (l * KC + kc) * JT + j
                        nc.tensor.matmul(
                            ps[:, j * bl:(j + 1) * bl],
                            whhT_s[:, base * 128:(base + 1) * 128],
                            prev_slab[:, kc * bl:(kc + 1) * bl],
                            start=(kc == 0), stop=(kc == KC - 1),
                        )
                if top:
                    ys = small.tile([128, fb], f32, tag="yslab")
                    out_slab = ys[:]
                else:
                    out_slab = y_r[:, t, :]
                for j in range(JT):
                    tmp = small.tile([128, bl], f32, tag="ztmp")
                    nc.vector.tensor_add(tmp[:], ps[:, j * bl:(j + 1) * bl],
                                         xw_r[:, t, j * bl:(j + 1) * bl])
                    nc.scalar.activation(out_slab[:, j * bl:(j + 1) * bl], tmp[:], Tanh)
                if top:
                    hT = ring.tile([128, fb], bf16, tag="hT")
                    nc.vector.tensor_copy(hT[:], ys[:])
                    nc.sync.dma_start(y_dst[t], ys[:])
                    prev_slab = hT[:]
                else:
                    prev_slab = y_r[:, t, :]
            if not top:
                yprev_r = y_r

    nc.compile()
    return nc


def _unfold_y(arr, nsteps=S, bl=BL):
    """[t, j*bl+b, p] fp32 -> [t, b, j*128+p]."""
    return np.ascontiguousarray(
        arr.reshape(nsteps, JT, bl, 128).transpose(0, 2, 1, 3).reshape(nsteps, bl, H)
    )


def _in_maps(input_x, w_ih_first, w_ih_rest, w_hh, b_ih, b_hh, nsteps=S, bl=BL, ncores=NCORES):
    wih0T = _pack_wT(np.asarray(w_ih_first))
    wihT = np.concatenate([_pack_wT(np.asarray(w_ih_rest)[i]) for i in range(NL - 1)], axis=1)
    whhT = np.concatenate([_pack_wT(np.asarray(w_hh)[i]) for i in range(NL)], axis=1)
    biasT = _pack_bias(b_ih, b_hh)
    maps = []
    for c in range(ncores):
        xs = np.asarray(input_x)[:nsteps, c * bl:(c + 1) * bl, :]
        maps.append({
            "xT": _pack_xT(xs),
            "wih0T": wih0T,
            "wihT": wihT,
            "whhT": whhT,
            "biasT": biasT,
        })
    return maps


def kernel(input_x, w_ih_first, w_ih_rest, w_hh, b_ih, b_hh):
    from concourse.bass_utils import run_bass_kernel_spmd

    key = (S, BL)
    if key not in _NC_CACHE:
        _NC_CACHE[key] = _build(S, BL)
    nc = _NC_CACHE[key]

    maps = _in_maps(input_x, w_ih_first, w_ih_rest, w_hh, b_ih, b_hh)
    res = run_bass_kernel_spmd(nc, maps, core_ids=list(range(NCORES)))
    outs = [_unfold_y(res.results[c]["y_out"]) for c in range(NCORES)]
    return np.concatenate(outs, axis=1).astype(np.float32)


# revision 14
# speedup vs baseline: 5.8783x; 5.8783x over previous
"""Trainium2 Bass kernel for a 3-layer tanh RNN (SEQ=512, BATCH=64, IN=128, HID=512).

Strategy (v1): data-parallel over batch — 8 NeuronCores, 8 batch rows each,
weights replicated. All compute on device:
  - per-layer input projection xw = y_prev @ W_ih^T + b   (big GEMMs, folded-T layout)
  - sequential 512-step recurrence h_t = tanh(xw_t + h_{t-1} @ W_hh^T)
Layouts keep h transposed+folded ([128 partitions, 4*BL cols] where row p of
j-block is hidden unit j*128+p) so no transposes are needed inside the scan.
Weights/activations bf16 (fp32 PSUM accumulation), xw and output fp32.
"""

import numpy as np
import ml_dtypes

S, B, IN, H, NL = 512, 64, 128, 512, 3
NCORES = 8
BL = B // NCORES          # batch rows per core
JT = H // 128             # output j-tiles
KC = H // 128             # contraction chunks
FB = JT * BL              # folded slab width per timestep

_NC_CACHE = {}


# ---------------- host-side packing ----------------

def _pack_wT(w):
    """[Hout, Hin] -> [128, (Hin/128)*(Hout/128)*128] bf16; tile (kc, j) at
    cols ((kc*JT + j)*128 : +128), lhsT[k=p, m] = w[j*128+m, kc*128+p]."""
    jt = w.shape[0] // 128
    kcn = w.shape[1] // 128
    w4 = np.asarray(w, np.float32).reshape(jt, 128, kcn, 128)
    return np.ascontiguousarray(
        w4.transpose(3, 2, 0, 1).reshape(128, kcn * jt * 128)
    ).astype(ml_dtypes.bfloat16)


def _pack_bias(b_ih, b_hh):
    """[NL, H] x2 -> [128, NL*JT] fp32, col l*JT+j row p = bias[l, j*128+p]."""
    s = np.asarray(b_ih, np.float32) + np.asarray(b_hh, np.float32)
    return np.ascontiguousarray(s.reshape(NL, JT, 128).transpose(2, 0, 1).reshape(128, NL * JT))


def _pack_xT(x_slice):
    """[S, BL, IN] -> [IN, S*BL] bf16 (t-major, b-minor columns)."""
    return np.ascontiguousarray(
        np.asarray(x_slice, np.float32).transpose(2, 0, 1).reshape(IN, -1)
    ).astype(ml_dtypes.bfloat16)


# ---------------- device program ----------------

def _build(nsteps=S, bl=BL):
    from contextlib import ExitStack
    from concourse import bacc, mybir
    import concourse.tile as tile

    f32, bf16 = mybir.dt.float32, mybir.dt.bfloat16
    Tanh = mybir.ActivationFunctionType.Tanh
    fb = JT * bl
    TBLK = min(512 // bl, nsteps)   # timesteps per GEMM n-block (N = TBLK*bl <= 512)
    NBLK = nsteps // TBLK

    nc = bacc.Bacc("TRN2", target_bir_lowering=False, debug=False, num_devices=NCORES)

    xT_d = nc.dram_tensor("xT", [IN, nsteps * bl], bf16, kind="ExternalInput")
    wih0T_d = nc.dram_tensor("wih0T", [IN, JT * 128], bf16, kind="ExternalInput")
    wihT_d = nc.dram_tensor("wihT", [128, (NL - 1) * KC * JT * 128], bf16, kind="ExternalInput")
    whhT_d = nc.dram_tensor("whhT", [128, NL * KC * JT * 128], bf16, kind="ExternalInput")
    bias_d = nc.dram_tensor("biasT", [128, NL * JT], f32, kind="ExternalInput")
    y_d = nc.dram_tensor("y_out", [128, nsteps * fb], f32, kind="ExternalOutput")
    YCH = min(64, nsteps)               # steps per output staging chunk

    with tile.TileContext(nc) as tc, ExitStack() as ctx:
        consts = ctx.enter_context(tc.tile_pool(name="consts", bufs=1))
        big = ctx.enter_context(tc.tile_pool(name="big", bufs=1))
        yts = ctx.enter_context(tc.tile_pool(name="yts", bufs=2))
        gpsum = ctx.enter_context(tc.tile_pool(name="gpsum", bufs=4, space="PSUM"))
        rpsum = ctx.enter_context(tc.tile_pool(name="rpsum", bufs=4, space="PSUM"))
        small = ctx.enter_context(tc.tile_pool(name="small", bufs=6))
        ring = ctx.enter_context(tc.tile_pool(name="ring", bufs=4))

        xT_s = consts.tile([IN, nsteps * bl], bf16)
        nc.sync.dma_start(xT_s[:], xT_d.ap())
        wih0T_s = consts.tile([IN, JT * 128], bf16)
        nc.sync.dma_start(wih0T_s[:], wih0T_d.ap())
        wihT_s = consts.tile([128, (NL - 1) * KC * JT * 128], bf16)
        nc.sync.dma_start(wihT_s[:], wihT_d.ap())
        whhT_s = consts.tile([128, NL * KC * JT * 128], bf16)
        nc.sync.dma_start(whhT_s[:], whhT_d.ap())
        bias_s = consts.tile([128, NL * JT], f32)
        nc.sync.dma_start(bias_s[:], bias_d.ap())
        zeros_s = consts.tile([128, fb], bf16)
        nc.vector.memset(zeros_s[:], 0.0)

        ystage_pool = ctx.enter_context(tc.tile_pool(name="ystage", bufs=2))

        yprev_r = None
        for l in range(NL):
            top = l == NL - 1
            nkc = 1 if l == 0 else KC

            # ---- input projection: xwT[p, t, j*bl+b] = (y_prev @ W_ih^T + b)[t, b, j*128+p]
            xwT = big.tile([128, nsteps * fb], f32, tag="xwT")
            xw_r = xwT[:].rearrange("p (t f) -> p t f", f=fb)
            for j in range(JT):
                for nb in range(NBLK):
                    ps = gpsum.tile([128, TBLK * bl], f32)
                    ps3 = ps[:].rearrange("p (t b) -> p t b", b=bl)
                    for kc in range(nkc):
                        if l == 0:
                            lhsT = wih0T_s[:, j * 128:(j + 1) * 128]
                            rhs = xT_s[:, nb * TBLK * bl:(nb + 1) * TBLK * bl]
                            out = ps[:]
                        else:
                            base = ((l - 1) * KC + kc) * JT + j
                            lhsT = wihT_s[:, base * 128:(base + 1) * 128]
                            rhs = yprev_r[:, nb * TBLK:(nb + 1) * TBLK, kc * bl:(kc + 1) * bl]
                            out = ps3
                        nc.tensor.matmul(out, lhsT, rhs, start=(kc == 0), stop=(kc == nkc - 1))
                    nc.vector.tensor_scalar_add(
                        out=xw_r[:, nb * TBLK:(nb + 1) * TBLK, j * bl:(j + 1) * bl],
                        in0=ps3,
                        scalar1=bias_s[:, l * JT + j:l * JT + j + 1],
                    )

            # ---- recurrence
            if not top:
                yT = yts.tile([128, nsteps * fb], bf16, tag="yT")
                y_r = yT[:].rearrange("p (t f) -> p t f", f=fb)
            prev_slab = zeros_s[:]
            ystage = None
            for t in range(nsteps):
                if top and t % YCH == 0:
                    ystage = ystage_pool.tile([128, YCH * fb], f32, tag="ystage")
                ps = rpsum.tile([128, fb], f32)
                for j in range(JT):
                    for i in range(KC):
                        kc = (j + i) % KC   # rotate so block kc is first needed at MM index ~4*kc
                        base = (l * KC + kc) * JT + j
                        nc.tensor.matmul(
                            ps[:, j * bl:(j + 1) * bl],
                            whhT_s[:, base * 128:(base + 1) * 128],
                            prev_slab[:, kc * bl:(kc + 1) * bl],
                            start=(i == 0), stop=(i == KC - 1),
                        )
                if top:
                    out_slab = ystage[:, (t % YCH) * fb:(t % YCH + 1) * fb]
                else:
                    out_slab = y_r[:, t, :]
                for j in range(JT):
                    tmp = small.tile([128, bl], f32, tag="ztmp")
                    nc.vector.tensor_add(tmp[:], ps[:, j * bl:(j + 1) * bl],
                                         xw_r[:, t, j * bl:(j + 1) * bl])
                    nc.scalar.activation(out_slab[:, j * bl:(j + 1) * bl], tmp[:], Tanh)
                if top:
                    hT = ring.tile([128, fb], bf16, tag="hT")
                    nc.vector.tensor_copy(hT[:], out_slab)
                    if t % YCH == YCH - 1:
                        c0 = (t // YCH) * YCH * fb
                        nc.sync.dma_start(y_d.ap()[:, c0:c0 + YCH * fb], ystage[:])
                    prev_slab = hT[:]
                else:
                    prev_slab = y_r[:, t, :]
            if not top:
                yprev_r = y_r

    nc.compile()
    return nc


def _unfold_y(arr, nsteps=S, bl=BL):
    """[p, t*fb + j*bl + b] fp32 -> [t, b, j*128+p]."""
    return np.ascontiguousarray(
        arr.reshape(128, nsteps, JT, bl).transpose(1, 3, 2, 0).reshape(nsteps, bl, H)
    )


def _in_maps(input_x, w_ih_first, w_ih_rest, w_hh, b_ih, b_hh, nsteps=S, bl=BL, ncores=NCORES):
    wih0T = _pack_wT(np.asarray(w_ih_first))
    wihT = np.concatenate([_pack_wT(np.asarray(w_ih_rest)[i]) for i in range(NL - 1)], axis=1)
    whhT = np.concatenate([_pack_wT(np.asarray(w_hh)[i]) for i in range(NL)], axis=1)
    biasT = _pack_bias(b_ih, b_hh)
    maps = []
    for c in range(ncores):
        xs = np.asarray(input_x)[:nsteps, c * bl:(c + 1) * bl, :]
        maps.append({
            "xT": _pack_xT(xs),
            "wih0T": wih0T,
            "wihT": wihT,
            "whhT": whhT,
            "biasT": biasT,
        })
    return maps


def kernel(input_x, w_ih_first, w_ih_rest, w_hh, b_ih, b_hh):
    from concourse.bass_utils import run_bass_kernel_spmd

    key = (S, BL)
    if key not in _NC_CACHE:
        _NC_CACHE[key] = _build(S, BL)
    nc = _NC_CACHE[key]

    maps = _in_maps(input_x, w_ih_first, w_ih_rest, w_hh, b_ih, b_hh)
    res = run_bass_kernel_spmd(nc, maps, core_ids=list(range(NCORES)))
    outs = [_unfold_y(res.results[c]["y_out"]) for c in range(NCORES)]
    return np.concatenate(outs, axis=1).astype(np.float32)
